# revision 34
# baseline (speedup 1.0000x reference)
"""Trainium2 Bass kernel for ConditionalLinearAttention.

Math (per batch element b, shapes hardcoded):
  xf  = x[b].reshape(256, 4096)
  cf  = cond_emb[b].reshape(512, 128)
  kv  = Wcond @ cf                      # (1024, 128)
  k   = softmax(kv[:512], per-row over the 128 cond positions)
  v   = kv[512:]
  ctx[h] = k_h @ v_h.T                  # (64, 64) per head h
  out = Wout @ apply(ctx) @ Wq @ xf + b_out

ctx is tiny and per-batch, so the whole attention folds into one per-batch
matrix W_comb = Wout @ ctxE @ Wq (256x256); the spatial dimension then sees
ONE (256x256)@(256x4096) GEMM. Sharding: data-parallel over batch, one batch
element per core.

The kernel is DMA-bandwidth bound (~5.9 MB of fp16 I/O per core against two
HWDGE queues sharing 16 DMA engines at ~400 GB/s aggregate), so the schedule
is built around the two queues:

  sync queue:   cf, wcond tiles (phase-1 critical), then x chunks
  scalar queue: wcond tiles, wq, wout, then x chunks
  both queues:  output chunks, alternating, once the input stream drains

All streams are fp16 (same bytes as bf16, 8x the mantissa). exp() for the
softmax runs as a Schraudolph bit-trick with a quadratic mantissa correction
on the vector+gpsimd engines (split halves), so the scalar engine issues no
activation ops at all -- no activation-table load/restore DMAs appear in the
measured window, and the scalar engine is free to act as the second DMA
queue. Softmax normalization is folded into the context rows (no on-chip
transpose anywhere).
"""

import os

import numpy as np

B = 8
C = 256
N_SPATIAL = 4096  # 64*64
P = 128
N_CORES = 8

EXP_MODE = os.environ.get("KERNEL_EXP", "act")  # schraudolph | act
WARM = int(os.environ.get("KERNEL_WARM", "18"))

_CACHE = {}
LAST_RESULTS = None  # BassKernelResults of the most recent run (for test.py)

# Schraudolph exp: i32 = int(z*K1 + K2); wtilde = bitcast_f32(i32) approximates
# exp(z) with a piecewise-linear mantissa; multiply by g(f) = 2^f/(1+f)
# (quadratic fit, f = mantissa/2^23) to correct it.
_LOG2E = 1.4426950408889634
_K1 = np.float32(_LOG2E * (1 << 23))
_K2 = np.float32(127.0 * (1 << 23))
_ff = np.linspace(0, 1, 1001)
_g = 2.0**_ff / (1.0 + _ff)
_P2, _P1, _P0 = [float(c) for c in np.polyfit(_ff, _g, 2)]
_C2 = np.float32(_P2 * 2.0**-46)  # coefficients rescaled to act on mantissa
_C1 = np.float32(_P1 * 2.0**-23)
_C0 = np.float32(_P0)


def _build_nc():
    import concourse.bacc as bacc
    import concourse.mybir as mybir
    import concourse.tile as tile

    fp32 = mybir.dt.float32
    f16 = mybir.dt.float16
    i32 = mybir.dt.int32
    Alu = mybir.AluOpType
    AF = mybir.ActivationFunctionType

    nc = bacc.Bacc("TRN2", target_bir_lowering=False, debug=False,
                   num_devices=N_CORES)

    # Weights arrive host-packed partition-major so every DMA moves long
    # contiguous per-partition rows (2-4 KB packets; short rows are
    # packet-overhead-bound at ~2x lower DMA throughput).
    x_t = nc.dram_tensor("x", [C, N_SPATIAL], f16, kind="ExternalInput").ap()
    cf_t = nc.dram_tensor("cfp", [P, 512], f16, kind="ExternalInput").ap()
    wc_t = nc.dram_tensor("wcp", [P, 4096], f16, kind="ExternalInput").ap()
    wqo_t = nc.dram_tensor("wqop", [P, 2048], f16, kind="ExternalInput").ap()  # [wq 0:1024 | wo 1024:2048]
    bias_t = nc.dram_tensor("bias", [256, 1], fp32, kind="ExternalInput").ap()
    out_t = nc.dram_tensor("out", [C, N_SPATIAL], f16, kind="ExternalOutput").ap()

    CW = 1024        # x / out chunk width (4 chunks over the spatial dim)
    NW = 512         # matmul moving width (one PSUM bank)

    with tile.TileContext(nc) as tc:
        with (
            tc.tile_pool(name="main", bufs=1) as mainp,
            tc.tile_pool(name="work", bufs=2) as workp,
            tc.tile_pool(name="outp", bufs=3) as outp,
            tc.tile_pool(name="ps", bufs=3, space="PSUM") as psp,
            tc.tile_pool(name="psO", bufs=5, space="PSUM") as psO,
        ):
            xr = x_t.rearrange("(ck p) n -> p ck n", p=P)        # (128, 2, 4096)
            cfr = cf_t.rearrange("p (ko m) -> p ko m", ko=4)     # (128, 4, 128)
            wcr = wc_t.rearrange("p (ko o) -> p ko o", ko=4)     # (128, 4, 1024)
            wqor = wqo_t.rearrange("p (h i c) -> p h i c", h=2, i=4)  # (128, 2, 4, 256)
            br = bias_t.rearrange("(mo p) one -> p mo one", p=P) # (128, 2, 1)
            outr = out_t.rearrange("(mo p) n -> p mo n", p=P)    # (128, 2, 4096)

            # warmup operand tiles first: junk matmuls must be runnable the
            # moment the engines clear the entry rendezvous
            wl = mainp.tile([P, P], mybir.dt.bfloat16)
            nc.gpsimd.memset(wl, 1.0)
            wz = mainp.tile([P, 512], mybir.dt.bfloat16)
            nc.vector.memset(wz, 0.5)

            # --- input DMAs: phase-1-critical tensors first, split across
            # both HWDGE queues (sync + scalar) so the streams run in parallel
            cf_sb = mainp.tile([P, 4, 128], f16)
            nc.sync.dma_start(cf_sb, cfr)
            # two column-half wcond DMAs: 4 KB contiguous per-partition runs
            # and ~256 descriptors each keep the DMA-engine pipelines full
            wcA = mainp.tile([P, 2, 1024], f16, tag="wcA")
            nc.sync.dma_start(wcA, wcr[:, 0:2, :])
            wcB = mainp.tile([P, 2, 1024], f16, tag="wcB")
            nc.scalar.dma_start(wcB, wcr[:, 2:4, :])
            wq_sb = mainp.tile([P, 4, 256], f16, tag="wq")
            nc.scalar.dma_start(wq_sb, wqor[:, 0, :, :])
            wo_sb = mainp.tile([P, 4, 256], f16, tag="wo")
            nc.scalar.dma_start(wo_sb, wqor[:, 1, :, :])

            # x chunks interleaved across the two queues for monotone arrival
            x_sb = []
            for cc in range(4):
                t = mainp.tile([P, 2, CW], f16, tag=f"x{cc}")
                eng = nc.sync if cc in (0, 2) else nc.scalar
                eng.dma_start(t, xr[:, :, CW * cc:CW * (cc + 1)])
                x_sb.append(t)

            # bias: 256 tiny strided descriptors -> keep on the gpsimd SWDGE
            bias_sb = mainp.tile([P, 2, 1], fp32)
            nc.gpsimd.dma_start(bias_sb, br)
            ones_sb = mainp.tile([P, 1], f16)
            nc.vector.memset(ones_sb, 1.0)
            maskc = mainp.tile([P, 1], i32)
            nc.vector.memset(maskc, 0x007FFFFF)

            # PE warmup: junk matmuls with no DMA deps fill the otherwise-idle
            # weight-arrival window so HAM unthrottles before the real matmuls
            def keep_warm(n):
                for _ in range(n):
                    pj = psO.tile([P, 512], fp32, tag="O")
                    nc.tensor.matmul(pj, wl, wz, start=True, stop=True)

            keep_warm(WARM)

            # --- phase 1: per-batch W_comb (256x256) ---
            # kvT (cond position m on partitions): v half first so its
            # PSUM->SBUF copy runs on the DVE while the k half still matmuls
            def wc_j(j):
                t = wcA if j < 2 else wcB
                return t[:, j % 2, :]

            pvv = psp.tile([P, 512], fp32, tag="p1")
            for j in range(4):
                nc.tensor.matmul(pvv, cf_sb[:, j, :], wc_j(j)[:, 512:1024],
                                 start=(j == 0), stop=(j == 3))
            vT = mainp.tile([P, 512], f16)
            nc.vector.tensor_copy(out=vT, in_=pvv)

            # kv-k in two column halves (separate PSUM tiles so exp on half 0
            # never blocks the half-1 matmuls) -- each exp half pipelines
            # behind its own accumulation
            pkv0 = psp.tile([P, 256], fp32, tag="p1")
            pkv1 = psp.tile([P, 256], fp32, tag="p1")
            pkvs = [pkv0, pkv1]
            ek = mainp.tile([P, 512], f16)
            expk = [ek[:, 0:256], ek[:, 256:512]]
            for half in range(2):
                hs = slice(256 * half, 256 * (half + 1))
                pkv = pkvs[half]
                for j in range(4):
                    nc.tensor.matmul(pkv, cf_sb[:, j, :],
                                     wc_j(j)[:, 256 * half:256 * (half + 1)],
                                     start=(j == 0), stop=(j == 3))
                if EXP_MODE == "act":
                    nc.scalar.activation(out=ek[:, hs], in_=pkv,
                                         func=AF.Exp)
                else:
                    it = workp.tile([P, 256], i32, tag=f"i32{half}")
                    nc.vector.tensor_scalar(out=it, in0=pkv,
                                            scalar1=float(_K1), scalar2=float(_K2),
                                            op0=Alu.mult, op1=Alu.add)
                    eng = nc.vector if half == 0 else nc.gpsimd
                    mf = workp.tile([P, 256], fp32, tag=f"mf{half}")
                    eng.tensor_scalar(out=mf, in0=it, scalar1=maskc,
                                      scalar2=None, op0=Alu.bitwise_and)
                    u1 = workp.tile([P, 256], fp32, tag=f"u1{half}")
                    eng.tensor_scalar(out=u1, in0=mf,
                                      scalar1=float(_C2), scalar2=float(_C1),
                                      op0=Alu.mult, op1=Alu.add)
                    u2 = workp.tile([P, 256], fp32, tag=f"u2{half}")
                    eng.tensor_mul(u2, u1, mf)
                    eng.scalar_tensor_tensor(out=ek[:, hs], in0=u2,
                                             scalar=float(_C0),
                                             in1=it.bitcast(fp32),
                                             op0=Alu.add, op1=Alu.mult)

            # Z columns, raw contexts, and 1/Z-folded Wq tiles, pipelined
            # per exp half: blocks 0,1 start as soon as exp half 0 lands.
            # 1/Z folds into the Wq rows (the contraction dim of A), keeping
            # the reciprocal off the pc->A critical path.
            pz01 = psp.tile([P, 2], fp32, tag="p1")
            pz23 = psp.tile([P, 2], fp32, tag="p1")
            rc = workp.tile([P, 4], fp32, tag="rc")
            wqn = mainp.tile([P, 4, 256], f16)
            pcs = []
            for half in range(2):
                pzh = pz01 if half == 0 else pz23
                for i in (2 * half, 2 * half + 1):
                    ekb = ek[:, 128 * i:128 * (i + 1)]
                    nc.tensor.matmul(pzh[:, (i % 2):(i % 2) + 1], ekb, ones_sb,
                                     start=True, stop=True)
                for i in (2 * half, 2 * half + 1):
                    ekb = ek[:, 128 * i:128 * (i + 1)]
                    pc = psp.tile([P, 128], fp32, tag="p1")
                    nc.tensor.matmul(pc, ekb, vT[:, 128 * i:128 * (i + 1)],
                                     start=True, stop=True)
                    pcb = workp.tile([P, 128], f16, tag=f"pc{i}")
                    if i % 2 == 0:
                        nc.vector.tensor_copy(out=pcb, in_=pc)
                    else:
                        nc.scalar.activation(out=pcb, in_=pc, func=AF.Identity,
                                             scale=1.0)
                    pcs.append(pcb)
                rch = rc[:, 2 * half:2 * half + 2]
                nc.vector.reciprocal(rch, pzh)
                for i in (2 * half, 2 * half + 1):
                    if i % 2 == 0:
                        nc.vector.tensor_scalar_mul(wqn[:, i, :], wq_sb[:, i, :],
                                                    rc[:, i:i + 1])
                    else:
                        nc.scalar.activation(out=wqn[:, i, :], in_=wq_sb[:, i, :],
                                             func=AF.Identity,
                                             scale=rc[:, i:i + 1])

            # A[he, c] = ctx_h.T @ (Wq/Z) per head, W_comb accumulation
            # interleaved per k-tile as each A tile lands
            A_sb = mainp.tile([P, 4, 256], f16)
            pw0 = psO.tile([P, 256], fp32, tag="O")
            pw1 = psO.tile([P, 256], fp32, tag="O")
            pw = [pw0, pw1]
            for i in range(4):
                pa = psp.tile([P, 256], fp32, tag="p1")
                for h in range(2):
                    rs = slice(64 * h, 64 * (h + 1))
                    nc.tensor.matmul(pa[rs, :], pcs[i][rs, 64 * h:64 * (h + 1)],
                                     wqn[rs, i, :], start=True, stop=True)
                if i % 2 == 0:
                    nc.vector.tensor_copy(out=A_sb[:, i, :], in_=pa)
                else:
                    nc.scalar.activation(out=A_sb[:, i, :], in_=pa,
                                         func=AF.Identity, scale=1.0)
                for mc in range(2):
                    nc.tensor.matmul(pw[mc], A_sb[:, i, 128 * mc:128 * (mc + 1)],
                                     wo_sb[:, i, :], start=(i == 0), stop=(i == 3),
                                     skip_group_check=True)

            wc_sb = mainp.tile([P, 2, 256], f16)
            nc.vector.tensor_copy(out=wc_sb[:, 0, :], in_=pw[0])
            nc.scalar.activation(out=wc_sb[:, 1, :], in_=pw[1],
                                 func=AF.Identity, scale=1.0)

            keep_warm(3)

            # --- phase 2: OUT = W_comb @ xf + bias, streamed over x chunks.
            # PSUM drain + bias add on vector/gpsimd; output DMAs alternate
            # between the two HWDGE queues, which are done with inputs by now.
            for cc in range(4):
                ot = outp.tile([P, 2, CW], f16, tag="osb")
                for sub in range(CW // NW):
                    for mo in range(2):
                        po = psO.tile([P, NW], fp32, tag="O")
                        for ck in range(2):
                            nc.tensor.matmul(
                                po, wc_sb[:, ck, 128 * mo:128 * (mo + 1)],
                                x_sb[cc][:, ck, NW * sub:NW * (sub + 1)],
                                start=(ck == 0), stop=(ck == 1))
                        if mo == 0:
                            nc.scalar.activation(
                                out=ot[:, mo, NW * sub:NW * (sub + 1)], in_=po,
                                func=AF.Identity, bias=bias_sb[:, mo, :],
                                scale=1.0)
                        else:
                            nc.vector.tensor_scalar_add(
                                out=ot[:, mo, NW * sub:NW * (sub + 1)], in0=po,
                                scalar1=bias_sb[:, mo, :])
                if cc < 3:
                    eng = nc.scalar if cc == 2 else nc.sync
                    eng.dma_start(outr[:, :, CW * cc:CW * (cc + 1)], ot)
                else:
                    nc.sync.dma_start(
                        outr[:, :, CW * cc:CW * cc + NW], ot[:, :, 0:NW])
                    nc.scalar.dma_start(
                        outr[:, :, CW * cc + NW:CW * (cc + 1)], ot[:, :, NW:CW])

    nc.compile()
    return nc


def kernel(x, cond_emb, Wq, Wcond, Wout, b_out):
    from concourse.bass_utils import run_bass_kernel_spmd

    global LAST_RESULTS

    if "nc" not in _CACHE:
        _CACHE["nc"] = _build_nc()
    nc = _CACHE["nc"]

    f16 = np.float16
    xf = np.ascontiguousarray(x.reshape(B, C, N_SPATIAL)).astype(f16)
    # partition-major packs: row p holds the 4 fold-slices [idx*128+p, :]
    # concatenated, so every DMA moves 2-4 KB contiguous per partition
    cf = cond_emb.reshape(B, 4, P, 128).transpose(0, 2, 1, 3).reshape(B, P, 512)
    cfp = np.ascontiguousarray(cf).astype(f16)
    wcp = np.ascontiguousarray(
        Wcond.T.reshape(4, P, 1024).transpose(1, 0, 2).reshape(P, 4096)
    ).astype(f16)
    wqop = np.ascontiguousarray(
        np.concatenate([Wq.reshape(4, P, 256).transpose(1, 0, 2),
                        Wout.T.reshape(4, P, 256).transpose(1, 0, 2)],
                       axis=1).reshape(P, 2048)
    ).astype(f16)
    bias = np.ascontiguousarray(b_out.reshape(256, 1)).astype(np.float32)

    in_maps = [
        {
            "x": np.ascontiguousarray(xf[b]),
            "cfp": np.ascontiguousarray(cfp[b]),
            "wcp": wcp,
            "wqop": wqop,
            "bias": bias,
        }
        for b in range(B)
    ]

    trace = bool(int(os.environ.get("KERNEL_TRACE", "0")))
    res = run_bass_kernel_spmd(nc, in_maps, core_ids=list(range(N_CORES)),
                               trace=trace)
    LAST_RESULTS = res
    out = np.stack([res.results[b]["out"] for b in range(B)])
    return out.reshape(B, C, 64, 64).astype(np.float32)


if __name__ == "__main__":
    xs = np.random.RandomState(0)
    ins = {
        "x": xs.randn(8, 256, 64, 64).astype(np.float32),
        "cond_emb": xs.randn(8, 512, 1, 128).astype(np.float32),
        "Wq": (xs.randn(512, 256) * 0.05).astype(np.float32),
        "Wcond": (xs.randn(1024, 512) * 0.05).astype(np.float32),
        "Wout": (xs.randn(256, 512) * 0.05).astype(np.float32),
        "b_out": np.zeros(256, np.float32),
    }
    o = kernel(**ins)
    print("ran, shape", o.shape)


# revision 35
# speedup vs baseline: 1.0288x; 1.0288x over previous
"""Trainium2 Bass kernel for ConditionalLinearAttention.

Math (per batch element b, shapes hardcoded):
  xf  = x[b].reshape(256, 4096)
  cf  = cond_emb[b].reshape(512, 128)
  kv  = Wcond @ cf                      # (1024, 128)
  k   = softmax(kv[:512], per-row over the 128 cond positions)
  v   = kv[512:]
  ctx[h] = k_h @ v_h.T                  # (64, 64) per head h
  out = Wout @ apply(ctx) @ Wq @ xf + b_out

ctx is tiny and per-batch, so the whole attention folds into one per-batch
matrix W_comb = Wout @ ctxE @ Wq (256x256); the spatial dimension then sees
ONE (256x256)@(256x4096) GEMM. Sharding: data-parallel over batch, one batch
element per core.

The kernel is DMA-bandwidth bound (~5.9 MB of fp16 I/O per core against two
HWDGE queues sharing 16 DMA engines at ~400 GB/s aggregate), so the schedule
is built around the two queues:

  sync queue:   cf, wcond tiles (phase-1 critical), then x chunks
  scalar queue: wcond tiles, wq, wout, then x chunks
  both queues:  output chunks, alternating, once the input stream drains

All streams are fp16 (same bytes as bf16, 8x the mantissa). exp() for the
softmax runs as a Schraudolph bit-trick with a quadratic mantissa correction
on the vector+gpsimd engines (split halves), so the scalar engine issues no
activation ops at all -- no activation-table load/restore DMAs appear in the
measured window, and the scalar engine is free to act as the second DMA
queue. Softmax normalization is folded into the context rows (no on-chip
transpose anywhere).
"""

import os

import numpy as np

B = 8
C = 256
N_SPATIAL = 4096  # 64*64
P = 128
N_CORES = 8

EXP_MODE = os.environ.get("KERNEL_EXP", "act")  # schraudolph | act
WARM = int(os.environ.get("KERNEL_WARM", "21"))

_CACHE = {}
LAST_RESULTS = None  # BassKernelResults of the most recent run (for test.py)

# Schraudolph exp: i32 = int(z*K1 + K2); wtilde = bitcast_f32(i32) approximates
# exp(z) with a piecewise-linear mantissa; multiply by g(f) = 2^f/(1+f)
# (quadratic fit, f = mantissa/2^23) to correct it.
_LOG2E = 1.4426950408889634
_K1 = np.float32(_LOG2E * (1 << 23))
_K2 = np.float32(127.0 * (1 << 23))
_ff = np.linspace(0, 1, 1001)
_g = 2.0**_ff / (1.0 + _ff)
_P2, _P1, _P0 = [float(c) for c in np.polyfit(_ff, _g, 2)]
_C2 = np.float32(_P2 * 2.0**-46)  # coefficients rescaled to act on mantissa
_C1 = np.float32(_P1 * 2.0**-23)
_C0 = np.float32(_P0)


def _build_nc():
    import concourse.bacc as bacc
    import concourse.mybir as mybir
    import concourse.tile as tile

    fp32 = mybir.dt.float32
    f16 = mybir.dt.float16
    i32 = mybir.dt.int32
    Alu = mybir.AluOpType
    AF = mybir.ActivationFunctionType

    nc = bacc.Bacc("TRN2", target_bir_lowering=False, debug=False,
                   num_devices=N_CORES)

    # Weights arrive host-packed partition-major so every DMA moves long
    # contiguous per-partition rows (2-4 KB packets; short rows are
    # packet-overhead-bound at ~2x lower DMA throughput).
    x_t = nc.dram_tensor("x", [C, N_SPATIAL], f16, kind="ExternalInput").ap()
    cf_t = nc.dram_tensor("cfp", [P, 512], f16, kind="ExternalInput").ap()
    wc_t = nc.dram_tensor("wcp", [P, 4096], f16, kind="ExternalInput").ap()
    wqo_t = nc.dram_tensor("wqop", [P, 2048], f16, kind="ExternalInput").ap()  # [wq 0:1024 | wo 1024:2048]
    bias_t = nc.dram_tensor("bias", [256, 1], fp32, kind="ExternalInput").ap()
    out_t = nc.dram_tensor("out", [C, N_SPATIAL], f16, kind="ExternalOutput").ap()

    CW = 1024        # x / out chunk width (4 chunks over the spatial dim)
    NW = 512         # matmul moving width (one PSUM bank)

    with tile.TileContext(nc) as tc:
        with (
            tc.tile_pool(name="main", bufs=1) as mainp,
            tc.tile_pool(name="work", bufs=2) as workp,
            tc.tile_pool(name="outp", bufs=3) as outp,
            tc.tile_pool(name="ps", bufs=3, space="PSUM") as psp,
            tc.tile_pool(name="psO", bufs=5, space="PSUM") as psO,
        ):
            xr = x_t.rearrange("(ck p) n -> p ck n", p=P)        # (128, 2, 4096)
            cfr = cf_t.rearrange("p (ko m) -> p ko m", ko=4)     # (128, 4, 128)
            wcr = wc_t.rearrange("p (ko o) -> p ko o", ko=4)     # (128, 4, 1024)
            wqor = wqo_t.rearrange("p (h i c) -> p h i c", h=2, i=4)  # (128, 2, 4, 256)
            br = bias_t.rearrange("(mo p) one -> p mo one", p=P) # (128, 2, 1)
            outr = out_t.rearrange("(mo p) n -> p mo n", p=P)    # (128, 2, 4096)

            # warmup operand tiles first: junk matmuls must be runnable the
            # moment the engines clear the entry rendezvous
            wl = mainp.tile([P, P], mybir.dt.bfloat16)
            nc.gpsimd.memset(wl, 1.0)
            wz = mainp.tile([P, 512], mybir.dt.bfloat16)
            nc.vector.memset(wz, 0.5)

            # --- input DMAs: phase-1-critical tensors first, split across
            # both HWDGE queues (sync + scalar) so the streams run in parallel
            cf_sb = mainp.tile([P, 4, 128], f16)
            nc.sync.dma_start(cf_sb, cfr)
            # two column-half wcond DMAs: 4 KB contiguous per-partition runs
            # and ~256 descriptors each keep the DMA-engine pipelines full
            wcA = mainp.tile([P, 2, 1024], f16, tag="wcA")
            nc.sync.dma_start(wcA, wcr[:, 0:2, :])
            wcB = mainp.tile([P, 2, 1024], f16, tag="wcB")
            nc.scalar.dma_start(wcB, wcr[:, 2:4, :])
            wq_sb = mainp.tile([P, 4, 256], f16, tag="wq")
            nc.scalar.dma_start(wq_sb, wqor[:, 0, :, :])
            wo_sb = mainp.tile([P, 4, 256], f16, tag="wo")
            nc.scalar.dma_start(wo_sb, wqor[:, 1, :, :])

            # x chunks interleaved across the two queues for monotone arrival
            x_sb = []
            for cc in range(4):
                t = mainp.tile([P, 2, CW], f16, tag=f"x{cc}")
                eng = nc.sync if cc in (0, 2) else nc.scalar
                eng.dma_start(t, xr[:, :, CW * cc:CW * (cc + 1)])
                x_sb.append(t)

            # bias: 256 tiny strided descriptors -> keep on the gpsimd SWDGE
            bias_sb = mainp.tile([P, 2, 1], fp32)
            nc.gpsimd.dma_start(bias_sb, br)
            ones_sb = mainp.tile([P, 1], f16)
            nc.vector.memset(ones_sb, 1.0)
            maskc = mainp.tile([P, 1], i32)
            nc.vector.memset(maskc, 0x007FFFFF)

            # PE warmup: junk matmuls with no DMA deps fill the otherwise-idle
            # weight-arrival window so HAM unthrottles before the real matmuls
            def keep_warm(n):
                for _ in range(n):
                    pj = psO.tile([P, 512], fp32, tag="O")
                    nc.tensor.matmul(pj, wl, wz, start=True, stop=True)

            keep_warm(WARM)

            # --- phase 1: per-batch W_comb (256x256) ---
            # kvT (cond position m on partitions): v half first so its
            # PSUM->SBUF copy runs on the DVE while the k half still matmuls
            def wc_j(j):
                t = wcA if j < 2 else wcB
                return t[:, j % 2, :]

            pvv = psp.tile([P, 512], fp32, tag="p1")
            for j in range(4):
                nc.tensor.matmul(pvv, cf_sb[:, j, :], wc_j(j)[:, 512:1024],
                                 start=(j == 0), stop=(j == 3))
            vT = mainp.tile([P, 512], f16)
            nc.vector.tensor_copy(out=vT, in_=pvv)

            # kv-k in two column halves (separate PSUM tiles so exp on half 0
            # never blocks the half-1 matmuls) -- each exp half pipelines
            # behind its own accumulation
            pkv0 = psp.tile([P, 256], fp32, tag="p1")
            pkv1 = psp.tile([P, 256], fp32, tag="p1")
            pkvs = [pkv0, pkv1]
            ek = mainp.tile([P, 512], f16)
            expk = [ek[:, 0:256], ek[:, 256:512]]
            for half in range(2):
                hs = slice(256 * half, 256 * (half + 1))
                pkv = pkvs[half]
                for j in range(4):
                    nc.tensor.matmul(pkv, cf_sb[:, j, :],
                                     wc_j(j)[:, 256 * half:256 * (half + 1)],
                                     start=(j == 0), stop=(j == 3))
                if EXP_MODE == "act":
                    nc.scalar.activation(out=ek[:, hs], in_=pkv,
                                         func=AF.Exp)
                else:
                    it = workp.tile([P, 256], i32, tag=f"i32{half}")
                    nc.vector.tensor_scalar(out=it, in0=pkv,
                                            scalar1=float(_K1), scalar2=float(_K2),
                                            op0=Alu.mult, op1=Alu.add)
                    eng = nc.vector if half == 0 else nc.gpsimd
                    mf = workp.tile([P, 256], fp32, tag=f"mf{half}")
                    eng.tensor_scalar(out=mf, in0=it, scalar1=maskc,
                                      scalar2=None, op0=Alu.bitwise_and)
                    u1 = workp.tile([P, 256], fp32, tag=f"u1{half}")
                    eng.tensor_scalar(out=u1, in0=mf,
                                      scalar1=float(_C2), scalar2=float(_C1),
                                      op0=Alu.mult, op1=Alu.add)
                    u2 = workp.tile([P, 256], fp32, tag=f"u2{half}")
                    eng.tensor_mul(u2, u1, mf)
                    eng.scalar_tensor_tensor(out=ek[:, hs], in0=u2,
                                             scalar=float(_C0),
                                             in1=it.bitcast(fp32),
                                             op0=Alu.add, op1=Alu.mult)

            # Z columns, raw contexts, and 1/Z-folded Wq tiles, pipelined
            # per exp half: blocks 0,1 start as soon as exp half 0 lands.
            # 1/Z folds into the Wq rows (the contraction dim of A), keeping
            # the reciprocal off the pc->A critical path.
            pz01 = psp.tile([P, 2], fp32, tag="p1")
            pz23 = psp.tile([P, 2], fp32, tag="p1")
            rc = workp.tile([P, 4], fp32, tag="rc")
            pcs = []
            for half in range(2):
                pzh = pz01 if half == 0 else pz23
                for i in (2 * half, 2 * half + 1):
                    ekb = ek[:, 128 * i:128 * (i + 1)]
                    nc.tensor.matmul(pzh[:, (i % 2):(i % 2) + 1], ekb, ones_sb,
                                     start=True, stop=True)
                rch = rc[:, 2 * half:2 * half + 2]
                nc.vector.reciprocal(rch, pzh)
                for i in (2 * half, 2 * half + 1):
                    ekb = ek[:, 128 * i:128 * (i + 1)]
                    pc = psp.tile([P, 128], fp32, tag="p1")
                    nc.tensor.matmul(pc, ekb, vT[:, 128 * i:128 * (i + 1)],
                                     start=True, stop=True)
                    pcb = workp.tile([P, 128], f16, tag=f"pc{i}")
                    if i % 2 == 0:
                        nc.vector.tensor_scalar_mul(pcb, pc, rc[:, i:i + 1])
                    else:
                        nc.scalar.activation(out=pcb, in_=pc, func=AF.Identity,
                                             scale=rc[:, i:i + 1])
                    pcs.append(pcb)

            # A[he, c] = ctx_h.T @ (Wq/Z) per head, W_comb accumulation
            # interleaved per k-tile as each A tile lands
            A_sb = mainp.tile([P, 4, 256], f16)
            pw0 = psO.tile([P, 256], fp32, tag="O")
            pw1 = psO.tile([P, 256], fp32, tag="O")
            pw = [pw0, pw1]
            for i in range(4):
                pa = psp.tile([P, 256], fp32, tag="p1")
                for h in range(2):
                    rs = slice(64 * h, 64 * (h + 1))
                    nc.tensor.matmul(pa[rs, :], pcs[i][rs, 64 * h:64 * (h + 1)],
                                     wq_sb[rs, i, :], start=True, stop=True)
                if i % 2 == 0:
                    nc.vector.tensor_copy(out=A_sb[:, i, :], in_=pa)
                else:
                    nc.scalar.activation(out=A_sb[:, i, :], in_=pa,
                                         func=AF.Identity, scale=1.0)
                for mc in range(2):
                    nc.tensor.matmul(pw[mc], A_sb[:, i, 128 * mc:128 * (mc + 1)],
                                     wo_sb[:, i, :], start=(i == 0), stop=(i == 3),
                                     skip_group_check=True)

            wc_sb = mainp.tile([P, 2, 256], f16)
            nc.vector.tensor_copy(out=wc_sb[:, 0, :], in_=pw[0])
            nc.scalar.activation(out=wc_sb[:, 1, :], in_=pw[1],
                                 func=AF.Identity, scale=1.0)

            # --- phase 2: OUT = W_comb @ xf + bias, streamed over x chunks.
            # PSUM drain + bias add on vector/gpsimd; output DMAs alternate
            # between the two HWDGE queues, which are done with inputs by now.
            for cc in range(4):
                ot = outp.tile([P, 2, CW], f16, tag="osb")
                for sub in range(CW // NW):
                    for mo in range(2):
                        po = psO.tile([P, NW], fp32, tag="O")
                        for ck in range(2):
                            nc.tensor.matmul(
                                po, wc_sb[:, ck, 128 * mo:128 * (mo + 1)],
                                x_sb[cc][:, ck, NW * sub:NW * (sub + 1)],
                                start=(ck == 0), stop=(ck == 1))
                        if mo == 0:
                            nc.scalar.activation(
                                out=ot[:, mo, NW * sub:NW * (sub + 1)], in_=po,
                                func=AF.Identity, bias=bias_sb[:, mo, :],
                                scale=1.0)
                        else:
                            nc.vector.tensor_scalar_add(
                                out=ot[:, mo, NW * sub:NW * (sub + 1)], in0=po,
                                scalar1=bias_sb[:, mo, :])
                if cc < 3:
                    eng = nc.scalar if cc == 2 else nc.sync
                    eng.dma_start(outr[:, :, CW * cc:CW * (cc + 1)], ot)
                else:
                    nc.sync.dma_start(
                        outr[:, :, CW * cc:CW * cc + NW], ot[:, :, 0:NW])
                    nc.scalar.dma_start(
                        outr[:, :, CW * cc + NW:CW * (cc + 1)], ot[:, :, NW:CW])

    nc.compile()
    return nc


def kernel(x, cond_emb, Wq, Wcond, Wout, b_out):
    from concourse.bass_utils import run_bass_kernel_spmd

    global LAST_RESULTS

    if "nc" not in _CACHE:
        _CACHE["nc"] = _build_nc()
    nc = _CACHE["nc"]

    f16 = np.float16
    xf = np.ascontiguousarray(x.reshape(B, C, N_SPATIAL)).astype(f16)
    # partition-major packs: row p holds the 4 fold-slices [idx*128+p, :]
    # concatenated, so every DMA moves 2-4 KB contiguous per partition
    cf = cond_emb.reshape(B, 4, P, 128).transpose(0, 2, 1, 3).reshape(B, P, 512)
    cfp = np.ascontiguousarray(cf).astype(f16)
    wcp = np.ascontiguousarray(
        Wcond.T.reshape(4, P, 1024).transpose(1, 0, 2).reshape(P, 4096)
    ).astype(f16)
    wqop = np.ascontiguousarray(
        np.concatenate([Wq.reshape(4, P, 256).transpose(1, 0, 2),
                        Wout.T.reshape(4, P, 256).transpose(1, 0, 2)],
                       axis=1).reshape(P, 2048)
    ).astype(f16)
    bias = np.ascontiguousarray(b_out.reshape(256, 1)).astype(np.float32)

    in_maps = [
        {
            "x": np.ascontiguousarray(xf[b]),
            "cfp": np.ascontiguousarray(cfp[b]),
            "wcp": wcp,
            "wqop": wqop,
            "bias": bias,
        }
        for b in range(B)
    ]

    trace = bool(int(os.environ.get("KERNEL_TRACE", "0")))
    res = run_bass_kernel_spmd(nc, in_maps, core_ids=list(range(N_CORES)),
                               trace=trace)
    LAST_RESULTS = res
    out = np.stack([res.results[b]["out"] for b in range(B)])
    return out.reshape(B, C, 64, 64).astype(np.float32)


if __name__ == "__main__":
    xs = np.random.RandomState(0)
    ins = {
        "x": xs.randn(8, 256, 64, 64).astype(np.float32),
        "cond_emb": xs.randn(8, 512, 1, 128).astype(np.float32),
        "Wq": (xs.randn(512, 256) * 0.05).astype(np.float32),
        "Wcond": (xs.randn(1024, 512) * 0.05).astype(np.float32),
        "Wout": (xs.randn(256, 512) * 0.05).astype(np.float32),
        "b_out": np.zeros(256, np.float32),
    }
    o = kernel(**ins)
    print("ran, shape", o.shape)
